# revision 13
# baseline (speedup 1.0000x reference)
"""Trainium2 Bass kernel for nn_EuclideanLoss.

Math (matches the oracle):
    y_t  = transpose(y, (0, 2, 1))                 # [B, N, D]
    pd   = sqrt(sum((x - y_t)^2, axis=-1))         # [B, N]
    dist = mean(pd, axis=0); dist[1:3] *= 1.5
    loss = mean(dist)

Strategy: data-parallel over batch — each of the 8 NeuronCores takes 4
batches and computes its pair distances pd[b, n] on device; the tiny [B, N]
result is gathered to the host, which finishes mean/scale/mean in float64.

The problem is DMA-bound (16MB of input per core at the ~379 GB/s HBM
per-NC QoS cap -> ~43us dense stream is the floor).  Both loads produce
address-sequential HBM descriptors:
  * y[b] ([64, 8192] row-major) loads FLAT into [128, 4096]: partition
    p = 2d + nh holds y[d, nh*4096 : (nh+1)*4096] — 16KB-contiguous
    descriptors.  (nh = which half of the batch's n-range)
  * x[b] loads as [128, 2, 32, 64] = (q, nh, c, d) with
    n = nh*4096 + q*32 + c — per-group c-range DMAs (2KB descriptors) so
    subs can begin before the whole batch lands.
Compute per batch (c-groups of 8 columns):
  PE   8 transposes y_v[:, c, :] ([128, 128]) -> PSUM yT[q, c, 2d+nh],
       aligning y to x's n-to-partition map.
  DVE  diff = x - yT  (yT read through a stride-permuted PSUM view)
  ACT  sq = Square(diff)
  DVE  reduce over d -> d2[p, b, g, nh, c]
Per batch: ACT pd_b = Sqrt(d2[:, b]) (overlaps the stream).

Tail: the DVE drains a full batch (9.3us serial) after that batch's data
lands, so the LAST batch is restructured into 4 independent quarter-units
(y-quarter [128, 1024] with p = u*64+d, then its x-quarter, one diff group
each).  Data->compute granularity drops from 4MB to 1MB, leaving only one
group's sub->square->reduce chain after the final input byte instead of
four.  Quarter n-map: n = t*2048 + u*1024 + p*8 + c.

Output o[b<3, p, g, nh, c] = pd[b, nh*4096 + p*32 + g*8 + c];
       o[3,   p, t, u,  c] = pd[3, t*2048 + u*1024 + p*8 + c]; host undoes it.
"""

import numpy as np

import concourse.bacc as bacc
import concourse.bass as bass
import concourse.mybir as mybir
import concourse.tile as tile
from concourse import masks
from concourse.bass_utils import run_bass_kernel_spmd

# Cap the semaphore universe walrus manages.  Measured effect: -6..7us vs
# the unflagged build — the compiled schedule loses the end-of-stream DMA
# straggle that used to leave DVE with an ~8us post-stream backlog.
import concourse.bass_utils as _bu

_orig_run_command = _bu.run_command

def _run_command_semcap(cmd, **kw):
    if isinstance(cmd, list) and cmd and "walrus_driver" in str(cmd[0]):
        cmd = list(cmd) + ["--max-sem-num=176"]
    return _orig_run_command(cmd, **kw)

_bu.run_command = _run_command_semcap

B, N, D = 32, 8192, 64
NCORES = 8
BL = B // NCORES        # 4 local batches per core
P = 128                 # SBUF partitions
NH = 2                  # n-halves per batch (partition interleave of y)
CPB = N // NH // P      # 32 consecutive x rows per partition per half
NG = 4                  # c-groups per batch
GC = CPB // NG          # 8 columns per group
NQ = 4                  # quarter-units for the last batch
QN = N // NQ            # 2048 n per quarter

F32 = mybir.dt.float32


def _build() -> bass.Bass:
    # Bacc (not plain Bass): its compile() pass splits sem waits across
    # event-semaphore instructions — TRN2 instructions hold at most one wait,
    # and this walrus build rejects multi-wait instructions outright.
    nc = bacc.Bacc("TRN2", target_bir_lowering=False, debug=False, num_devices=NCORES)
    x_d = nc.dram_tensor("x", [BL, N, D], F32, kind="ExternalInput")
    y_d = nc.dram_tensor("y", [BL, D, N], F32, kind="ExternalInput")
    o_d = nc.dram_tensor("o", [P, BL, NG, NH, GC], F32, kind="ExternalOutput")
    nc.dram_tensor("cachebust_v3_quarter_tail", [1, 1], F32, kind="Internal")

    with tile.TileContext(nc) as tc:
        with (
            tc.tile_pool(name="const", bufs=1) as cpool,
            tc.tile_pool(name="io", bufs=4) as iopool,
            tc.tile_pool(name="work", bufs=4) as wpool,
            tc.tile_pool(name="psum", bufs=4, space="PSUM") as ppool,
        ):
            ident = cpool.tile([P, P], F32)
            masks.make_identity(nc, ident[:])
            d2a = cpool.tile([P, BL, NG, NH, GC], F32)
            pda = cpool.tile([P, BL, NG, NH, GC], F32)
            # Warm the Sqrt LUT during the DMA fill so the per-batch sqrts
            # do not stall ~1.3us on a lazy ACT_TABLE_LOAD.
            warm = cpool.tile([P, 1], F32)
            nc.scalar.activation(
                warm[:], ident[:, 0:1], mybir.ActivationFunctionType.Sqrt
            )

            def compute_group(x_ap, yT_v, b, g):
                """sub -> square -> reduce for one [P, NH, GC, D] group."""
                diff = wpool.tile([P, NH, GC, D], F32, tag="diff")
                nc.vector.tensor_sub(diff[:], x_ap, yT_v)
                sq = wpool.tile([P, NH, GC, D], F32, tag="sq")
                nc.scalar.activation(
                    sq[:], diff[:], mybir.ActivationFunctionType.Square
                )
                nc.vector.tensor_reduce(
                    d2a[:, b, g, :, :],
                    sq[:],
                    axis=mybir.AxisListType.X,
                    op=mybir.AluOpType.add,
                )

            for b in range(BL - 1):
                x_t = iopool.tile([P, NH, CPB, D], F32, tag="x")
                y_t = iopool.tile([P, NH * CPB * D], F32, tag="y")
                # y first: the transposes depend only on y, so PE can start
                # while x is still streaming in.
                nc.sync.dma_start(
                    y_t[:], y_d[b].rearrange("d (nh n) -> (d nh) n", nh=NH)
                )
                xsrc = x_d[b].rearrange("(nh q c) d -> q nh c d", nh=NH, c=CPB)
                for g in range(NG):
                    nc.sync.dma_start(
                        x_t[:, :, g * GC : (g + 1) * GC, :],
                        xsrc[:, :, g * GC : (g + 1) * GC, :],
                    )

                # column q of slice c holds n-offset q*32+c within each half
                y_v = y_t[:].rearrange("p (q c) -> p c q", c=CPB)
                for g in range(NG):
                    yT = ppool.tile([P, GC, P], F32, tag="yT")
                    for c in range(GC):
                        nc.tensor.transpose(
                            yT[:, c, :], y_v[:, g * GC + c, :], ident[:]
                        )
                    compute_group(
                        x_t[:, :, g * GC : (g + 1) * GC, :],
                        yT[:].rearrange("p c (d nh) -> p nh c d", nh=NH),
                        b,
                        g,
                    )
                nc.scalar.activation(
                    pda[:, b], d2a[:, b], mybir.ActivationFunctionType.Sqrt
                )

            # Last batch: 4 quarter-units, y-quarter then x-quarter each, so
            # the post-stream DVE backlog is one group, not four.
            b = BL - 1
            for t in range(NQ):
                yq = iopool.tile([P, QN * D // P], F32, tag="yq")
                # [128, 1024]: partition u*64+d holds y[d, t*2048+u*1024 : +1024]
                # (one DMA per u-half: contiguous partition block, 4KB descs)
                for u in range(NH):
                    nc.sync.dma_start(
                        yq[u * D : (u + 1) * D, :],
                        y_d[b, :, t * QN + u * (QN // NH) : t * QN + (u + 1) * (QN // NH)],
                    )
                xq = iopool.tile([P, NH, GC, D], F32, tag="xq")
                nc.sync.dma_start(
                    xq[:],
                    x_d[b, t * QN : (t + 1) * QN].rearrange(
                        "(u q c) d -> q u c d", u=NH, c=GC
                    ),
                )
                yq_v = yq[:].rearrange("p (q c) -> p c q", c=GC)
                yT = ppool.tile([P, GC, P], F32, tag="yT")
                for c in range(GC):
                    nc.tensor.transpose(yT[:, c, :], yq_v[:, c, :], ident[:])
                compute_group(
                    xq[:],
                    yT[:].rearrange("p c (nh d) -> p nh c d", nh=NH),
                    b,
                    t,
                )
            nc.scalar.activation(
                pda[:, b], d2a[:, b], mybir.ActivationFunctionType.Sqrt
            )

            # One contiguous store for all batches: per-batch strided stores
            # measured ~13us slower (they interleave small descriptors into
            # the input stream).
            nc.sync.dma_start(o_d[:], pda[:])
    nc.finalize()
    return nc


_NC_CACHE: list = []


def _get_program() -> bass.Bass:
    if not _NC_CACHE:
        _NC_CACHE.append(_build())
    return _NC_CACHE[0]


def kernel(x: np.ndarray, y: np.ndarray) -> np.ndarray:
    x = np.ascontiguousarray(np.asarray(x, dtype=np.float32))
    y = np.ascontiguousarray(np.asarray(y, dtype=np.float32))
    assert x.shape == (B, N, D) and y.shape == (B, D, N)

    nc = _get_program()
    in_maps = [
        {"x": x[i * BL : (i + 1) * BL], "y": y[i * BL : (i + 1) * BL]}
        for i in range(NCORES)
    ]
    res = run_bass_kernel_spmd(nc, in_maps, list(range(NCORES)))
    o = np.stack([res.results[i]["o"] for i in range(NCORES)])  # [8, P, BL, NG, NH, GC]
    pd = np.empty((B, N), dtype=np.float32)
    for core in range(NCORES):
        v = o[core]  # [P, BL, NG, NH, GC]
        for b in range(BL - 1):
            # o[p, b, g, nh, c] = pd[b, nh*4096 + p*32 + g*8 + c]
            pd[core * BL + b] = v[:, b].transpose(2, 0, 1, 3).reshape(N)
        # o[p, 3, t, u, c] = pd[3, t*2048 + u*1024 + p*8 + c]
        pd[core * BL + BL - 1] = v[:, BL - 1].transpose(1, 2, 0, 3).reshape(N)

    dist = pd.mean(axis=0, dtype=np.float64)
    dist[1:3] *= 1.5
    return np.asarray(dist.mean(), dtype=np.float32)


# revision 16
# speedup vs baseline: 1.1427x; 1.1427x over previous
"""Trainium2 Bass kernel for nn_EuclideanLoss.

Math (matches the oracle):
    y_t  = transpose(y, (0, 2, 1))                 # [B, N, D]
    pd   = sqrt(sum((x - y_t)^2, axis=-1))         # [B, N]
    dist = mean(pd, axis=0); dist[1:3] *= 1.5
    loss = mean(dist)

Strategy: data-parallel over batch — each of the 8 NeuronCores takes 4
batches and computes its pair distances pd[b, n] on device; the tiny [B, N]
result is gathered to the host, which finishes mean/scale/mean in float64.

The problem is DMA-bound (16MB of input per core at the ~379 GB/s HBM
per-NC QoS cap -> ~43us dense stream is the floor).  Both loads produce
address-sequential HBM descriptors:
  * y[b] ([64, 8192] row-major) loads FLAT into [128, 4096]: partition
    p = 2d + nh holds y[d, nh*4096 : (nh+1)*4096] — 16KB-contiguous
    descriptors.  (nh = which half of the batch's n-range)
  * x[b] loads as [128, 2, 32, 64] = (q, nh, c, d) with
    n = nh*4096 + q*32 + c — per-group c-range DMAs (2KB descriptors) so
    subs can begin before the whole batch lands.
Compute per batch (c-groups of 8 columns):
  PE   8 transposes y_v[:, c, :] ([128, 128]) -> PSUM yT[q, c, 2d+nh],
       aligning y to x's n-to-partition map.
  DVE  diff = x - yT  (yT read through a stride-permuted PSUM view)
  ACT  sq = Square(diff)
  DVE  reduce over d -> d2[p, b, g, nh, c]
Per batch: ACT pd_b = Sqrt(d2[:, b]) (overlaps the stream).

Tail: the DVE drains a full batch (9.3us serial) after that batch's data
lands, so the LAST batch is restructured into 4 independent quarter-units
(quarter t = the t-th 1024-column slice of BOTH n-halves, preserving the
uniform (d nh) partition map and full 128-partition DMA rate).  Data ->
compute granularity drops from 4MB to 1MB, leaving only one group's
sub->square->reduce chain after the final input byte instead of four.
Quarter n-map: n = nh*4096 + t*1024 + p*8 + c.

Output o[b<3, p, g, nh, c] = pd[b, nh*4096 + p*32 + g*8 + c];
       o[3,   p, t, nh, c] = pd[3, nh*4096 + t*1024 + p*8 + c]; host undoes it.
"""

import numpy as np

import concourse.bacc as bacc
import concourse.bass as bass
import concourse.mybir as mybir
import concourse.tile as tile
from concourse import masks
from concourse.bass_utils import run_bass_kernel_spmd

# Cap the semaphore universe walrus manages.  Measured effect: -6..7us vs
# the unflagged build — the compiled schedule loses the end-of-stream DMA
# straggle that used to leave DVE with an ~8us post-stream backlog.
import concourse.bass_utils as _bu

_orig_run_command = _bu.run_command

def _run_command_semcap(cmd, **kw):
    if isinstance(cmd, list) and cmd and "walrus_driver" in str(cmd[0]):
        cmd = list(cmd) + ["--max-sem-num=176"]
    return _orig_run_command(cmd, **kw)

_bu.run_command = _run_command_semcap

B, N, D = 32, 8192, 64
NCORES = 8
BL = B // NCORES        # 4 local batches per core
P = 128                 # SBUF partitions
NH = 2                  # n-halves per batch (partition interleave of y)
CPB = N // NH // P      # 32 consecutive x rows per partition per half
NG = 4                  # c-groups per batch
GC = CPB // NG          # 8 columns per group
NQ = 4                  # quarter-units for the last batch
QN = N // NQ            # 2048 n per quarter

F32 = mybir.dt.float32


def _build() -> bass.Bass:
    # Bacc (not plain Bass): its compile() pass splits sem waits across
    # event-semaphore instructions — TRN2 instructions hold at most one wait,
    # and this walrus build rejects multi-wait instructions outright.
    nc = bacc.Bacc("TRN2", target_bir_lowering=False, debug=False, num_devices=NCORES)
    x_d = nc.dram_tensor("x", [BL, N, D], F32, kind="ExternalInput")
    y_d = nc.dram_tensor("y", [BL, D, N], F32, kind="ExternalInput")
    o_d = nc.dram_tensor("o", [P, BL, NG, NH, GC], F32, kind="ExternalOutput")
    nc.dram_tensor("cachebust_v3_quarter_tail", [1, 1], F32, kind="Internal")

    with tile.TileContext(nc) as tc:
        with (
            tc.tile_pool(name="const", bufs=1) as cpool,
            tc.tile_pool(name="io", bufs=4) as iopool,
            tc.tile_pool(name="work", bufs=4) as wpool,
            tc.tile_pool(name="psum", bufs=4, space="PSUM") as ppool,
        ):
            ident = cpool.tile([P, P], F32)
            masks.make_identity(nc, ident[:])
            d2a = cpool.tile([P, BL, NG, NH, GC], F32)
            pda = cpool.tile([P, BL, NG, NH, GC], F32)
            # Warm the Sqrt LUT during the DMA fill so the per-batch sqrts
            # do not stall ~1.3us on a lazy ACT_TABLE_LOAD.
            warm = cpool.tile([P, 1], F32)
            nc.scalar.activation(
                warm[:], ident[:, 0:1], mybir.ActivationFunctionType.Sqrt
            )

            def compute_group(x_ap, yT_v, b, g):
                """sub -> square -> reduce for one [P, NH, GC, D] group."""
                diff = wpool.tile([P, NH, GC, D], F32, tag="diff")
                nc.vector.tensor_sub(diff[:], x_ap, yT_v)
                sq = wpool.tile([P, NH, GC, D], F32, tag="sq")
                nc.scalar.activation(
                    sq[:], diff[:], mybir.ActivationFunctionType.Square
                )
                nc.vector.tensor_reduce(
                    d2a[:, b, g, :, :],
                    sq[:],
                    axis=mybir.AxisListType.X,
                    op=mybir.AluOpType.add,
                )

            for b in range(BL - 1):
                x_t = iopool.tile([P, NH, CPB, D], F32, tag="x")
                y_t = iopool.tile([P, NH * CPB * D], F32, tag="y")
                # y first: the transposes depend only on y, so PE can start
                # while x is still streaming in.
                nc.sync.dma_start(
                    y_t[:], y_d[b].rearrange("d (nh n) -> (d nh) n", nh=NH)
                )
                xsrc = x_d[b].rearrange("(nh q c) d -> q nh c d", nh=NH, c=CPB)
                for g in range(NG):
                    nc.sync.dma_start(
                        x_t[:, :, g * GC : (g + 1) * GC, :],
                        xsrc[:, :, g * GC : (g + 1) * GC, :],
                    )

                # column q of slice c holds n-offset q*32+c within each half
                y_v = y_t[:].rearrange("p (q c) -> p c q", c=CPB)
                for g in range(NG):
                    yT = ppool.tile([P, GC, P], F32, tag="yT")
                    for c in range(GC):
                        nc.tensor.transpose(
                            yT[:, c, :], y_v[:, g * GC + c, :], ident[:]
                        )
                    compute_group(
                        x_t[:, :, g * GC : (g + 1) * GC, :],
                        yT[:].rearrange("p c (d nh) -> p nh c d", nh=NH),
                        b,
                        g,
                    )
                nc.scalar.activation(
                    pda[:, b], d2a[:, b], mybir.ActivationFunctionType.Sqrt
                )

            # Last batch: 4 quarter-units (quarter t = the t-th 1024-column
            # slice of BOTH n-halves, so the partition map stays the uniform
            # full-rate (d nh) layout), y-quarter then x-quarter each, so the
            # post-stream DVE backlog is one group, not four.
            b = BL - 1
            yqsrc = y_d[b].rearrange("d (nh t j) -> t (d nh) j", nh=NH, t=NQ)
            xqsrc = x_d[b].rearrange("(nh t q c) d -> t q nh c d", nh=NH, t=NQ, c=GC)
            for t in range(NQ):
                yq = iopool.tile([P, N * D // NQ // P], F32, tag="yq")
                nc.sync.dma_start(yq[:], yqsrc[t])
                xq = iopool.tile([P, NH, GC, D], F32, tag="xq")
                nc.sync.dma_start(xq[:], xqsrc[t])
                yq_v = yq[:].rearrange("p (q c) -> p c q", c=GC)
                yT = ppool.tile([P, GC, P], F32, tag="yT")
                for c in range(GC):
                    nc.tensor.transpose(yT[:, c, :], yq_v[:, c, :], ident[:])
                compute_group(
                    xq[:],
                    yT[:].rearrange("p c (d nh) -> p nh c d", nh=NH),
                    b,
                    t,
                )
            nc.scalar.activation(
                pda[:, b], d2a[:, b], mybir.ActivationFunctionType.Sqrt
            )

            # One contiguous store for all batches: per-batch strided stores
            # measured ~13us slower (they interleave small descriptors into
            # the input stream).
            nc.sync.dma_start(o_d[:], pda[:])
    nc.finalize()
    return nc


_NC_CACHE: list = []


def _get_program() -> bass.Bass:
    if not _NC_CACHE:
        _NC_CACHE.append(_build())
    return _NC_CACHE[0]


def kernel(x: np.ndarray, y: np.ndarray) -> np.ndarray:
    x = np.ascontiguousarray(np.asarray(x, dtype=np.float32))
    y = np.ascontiguousarray(np.asarray(y, dtype=np.float32))
    assert x.shape == (B, N, D) and y.shape == (B, D, N)

    nc = _get_program()
    in_maps = [
        {"x": x[i * BL : (i + 1) * BL], "y": y[i * BL : (i + 1) * BL]}
        for i in range(NCORES)
    ]
    res = run_bass_kernel_spmd(nc, in_maps, list(range(NCORES)))
    o = np.stack([res.results[i]["o"] for i in range(NCORES)])  # [8, P, BL, NG, NH, GC]
    pd = np.empty((B, N), dtype=np.float32)
    for core in range(NCORES):
        v = o[core]  # [P, BL, NG, NH, GC]
        for b in range(BL - 1):
            # o[p, b, g, nh, c] = pd[b, nh*4096 + p*32 + g*8 + c]
            pd[core * BL + b] = v[:, b].transpose(2, 0, 1, 3).reshape(N)
        # o[p, 3, t, nh, c] = pd[3, nh*4096 + t*1024 + p*8 + c]
        pd[core * BL + BL - 1] = v[:, BL - 1].transpose(2, 1, 0, 3).reshape(N)

    dist = pd.mean(axis=0, dtype=np.float64)
    dist[1:3] *= 1.5
    return np.asarray(dist.mean(), dtype=np.float32)
